# revision 31
# baseline (speedup 1.0000x reference)
"""Bass/Tile kernel for masked dot-product attention on 8 Trainium2 NeuronCores.

Problem: B=64, NQ=NK=1024, D=128, float32.
  scores = Q @ K^T / sqrt(D); mask keys >= valid_len[b] to -1e6;
  out = softmax(scores) @ V

Strategy (data parallel over batch, 8 batches per core):
  - Scores are computed TRANSPOSED per k-tile: s_T[k, q] = (K Q^T)[k, q] via
    matmul(lhsT=K^T tile [d,128k], rhs=Q^T [d,512q]).  With keys on the
    partition axis, the per-batch key mask becomes a per-partition bias on
    the exp activation: exp(s*scale + bias), bias in {0, -1e6}.
  - Softmax without max-subtraction (scores/sqrt(d) are ~N(0,1); exp never
    overflows; masked lanes underflow to exactly 0).
  - Phase 2 needs no transpose: out^T[d, q] = sum_k V[k,d] * e[k,q] via
    matmul(lhsT=V tile [k,d] (native layout), rhs=e[k,512q]); the softmax
    denominator rides on the PE too: den[*, q] = sum_k 1 * e[k,q] via an
    all-ones [128,128] stationary matrix, which also replicates the
    denominator across partitions so the final normalize is an elementwise
    tensor_tensor multiply.
  - Matmuls run in float32r (e8m11, 1 PE cycle/row at N=512 vs 4 for fp32);
    inputs are pre-rounded to the e8m11 grid on the host.
  - Fully-masked k-tiles are skipped entirely (matmul+exp+phase2).  Batches
    are sorted by ceil(valid/128) and dealt into 8 slots x 8 cores so the
    SPMD program (one instruction stream for all cores) uses the per-slot
    max k-tile count.
  - All per-slot inputs are packed host-side into one [128, W] blob so each
    slot loads with a single large fully-contiguous-per-partition DMA;
    section loads and output stores rotate across the three DMA-issuing
    engines (sync/scalar HWDGE rings + gpsimd SWDGE).
"""

import os
from contextlib import ExitStack

import numpy as np

import concourse.bacc as bacc
import concourse.tile as tile
from concourse import mybir
from concourse import bass_utils

B, NQ, NK, D = 64, 1024, 1024, 128
N_CORES = 8
SLOTS = B // N_CORES  # 8 batches per core
P = 128               # partition count == k-tile size
KT_MAX = NK // P      # 8 k-tiles per batch
QCH = 512             # q chunk (psum bank / fp32 matmul free-dim limit)
NQC = NQ // QCH       # 2 q chunks
SCALE = 1.0 / float(np.sqrt(D))
MASK_BIAS = -1.0e6

F32 = mybir.dt.float32
_E_DT_MAP = {
    "f32": mybir.dt.float32,
    "f16": mybir.dt.float16,
    "bf16": mybir.dt.bfloat16,
}
# float32r runs the PE at 1 cycle/row (vs 4 for plain float32) when the
# moving free dim is >= 256; all matmuls here use N=512.
MM_DT = mybir.dt.float32 if os.environ.get("ATTN_MM_F32") else mybir.dt.float32r
E_DT = mybir.dt.float16  # attention weights + V: fp16 (ScalarE 2x accel, half DMA)
# Q/K dtype: fp16 halves the blob DMA and enables fast weight load on the PE.
# Default fp16; set ATTN_QK32R=1 for the float32r path (e8m11 inputs).
def _qk_dt():
    return MM_DT if os.environ.get("ATTN_QK32R") else mybir.dt.float16


QK_DT = _qk_dt()


def _section_cols(nkt):
    """Column layout of one slot's section: [qt | kt] (f32r cols)."""
    return NQ + nkt * P


def _vsection_cols(nkt):
    """fp16 cols of one slot's v section."""
    return nkt * D


def _offsets(nkt_slots):
    offs = []
    voffs = []
    o = 0
    vo = 0
    for s in range(SLOTS):
        offs.append(o)
        voffs.append(vo)
        o += _section_cols(nkt_slots[s])
        vo += _vsection_cols(nkt_slots[s])
    return offs, o, voffs, vo  # f32r cols, fp16 cols


def build_program(nkt_slots, reps=1, probe=""):
    """Build the SPMD program for one core (8 slots with static k-tile counts)."""
    nc = bacc.Bacc("TRN2", target_bir_lowering=False, debug=False)

    global QK_DT
    QK_DT = _qk_dt()
    offs, W, voffs, VW = _offsets(nkt_slots)
    blob_d = nc.dram_tensor("blob", [P, W], QK_DT, kind="ExternalInput").ap()
    vblob_d = nc.dram_tensor("vblob", [P, VW], E_DT, kind="ExternalInput").ap()
    bias_d = nc.dram_tensor("bias", [P, SLOTS, KT_MAX], F32, kind="ExternalInput").ap()
    ones_d = nc.dram_tensor("ones", [P, P], E_DT, kind="ExternalInput").ap()
    out16 = not os.environ.get("ATTN_OUT32")
    out_d = nc.dram_tensor(
        "out_t", [SLOTS, P, NQ], E_DT if out16 else F32, kind="ExternalOutput"
    ).ap()

    with tile.TileContext(nc) as tc:
        with ExitStack() as ctx:
            ENGS = [nc.sync, nc.scalar, nc.gpsimd]
            const_pool = ctx.enter_context(tc.tile_pool(name="const", bufs=1))
            sec_pool = ctx.enter_context(tc.tile_pool(name="sec", bufs=1))
            e_bufs = 10 if os.environ.get("ATTN_DEN_END") else int(
                os.environ.get("ATTN_E_BUFS", "3")
            )
            e_pool = ctx.enter_context(tc.tile_pool(name="exp", bufs=e_bufs))
            ev_pool = ctx.enter_context(tc.tile_pool(name="evict", bufs=2))
            narrow_s = bool(os.environ.get("ATTN_NARROW_S"))
            sb1 = bool(os.environ.get("ATTN_SB1"))
            s16 = bool(os.environ.get("ATTN_S16"))
            s_bufs = 1 if sb1 else (3 if narrow_s else 2)
            od_bufs = 3 if (sb1 or s16) else (3 if narrow_s else 2)
            s_pool = ctx.enter_context(
                tc.tile_pool(name="spsum", bufs=s_bufs, space="PSUM")
            )
            o_pool = ctx.enter_context(
                tc.tile_pool(name="opsum", bufs=od_bufs, space="PSUM")
            )
            d_pool = ctx.enter_context(
                tc.tile_pool(
                    name="dpsum", bufs=od_bufs if (sb1 or s16) else 2, space="PSUM"
                )
            )

            ones_t = const_pool.tile([P, P], E_DT)
            nc.sync.dma_start(ones_t[:], ones_d[:])
            ones_r = ones_t[:]

            def body():
                bias_all = sec_pool.tile(
                    [P, SLOTS, KT_MAX], F32, tag="bias", name="bias_all"
                )
                nc.sync.dma_start(bias_all[:], bias_d[:])
                secs = []
                vsecs = []
                split3 = bool(os.environ.get("ATTN_SPLIT3"))
                for s in range(SLOTS):
                    w = _section_cols(nkt_slots[s])
                    sec_t = sec_pool.tile([P, w], QK_DT, tag=f"sec{s}", name=f"sec{s}")
                    if split3:
                        h = (w // 2 + 3) & ~3
                        ENGS[s % 3].dma_start(
                            sec_t[:, 0:h], blob_d[:, offs[s] : offs[s] + h]
                        )
                        ENGS[(s + 2) % 3].dma_start(
                            sec_t[:, h:w], blob_d[:, offs[s] + h : offs[s] + w]
                        )
                    else:
                        ENGS[s % 3].dma_start(
                            sec_t[:], blob_d[:, offs[s] : offs[s] + w]
                        )
                    secs.append(sec_t)
                    vw = _vsection_cols(nkt_slots[s])
                    vsec_t = sec_pool.tile(
                        [P, vw], E_DT, tag=f"vsec{s}", name=f"vsec{s}"
                    )
                    ENGS[(s + 1) % 3].dma_start(
                        vsec_t[:], vblob_d[:, voffs[s] : voffs[s] + vw]
                    )
                    vsecs.append(vsec_t)
                if probe == "mm":
                    # pure matmul throughput: all slots compute on slot-0 data
                    nkt0 = nkt_slots[0]
                    sec0 = secs[0]
                    qt0 = sec0[:, 0:NQ]
                    kt0 = sec0[:, NQ : NQ + nkt0 * P]
                    for s in range(SLOTS):
                        for kti in range(nkt0):
                            s_full = s_pool.tile([P, NQ], F32, tag="s", name="s_ps")
                            for qc in range(NQC):
                                nc.tensor.matmul(
                                    s_full[:, qc * QCH : (qc + 1) * QCH],
                                    kt0[:, kti * P : (kti + 1) * P],
                                    qt0[:, qc * QCH : (qc + 1) * QCH],
                                    start=True,
                                    stop=True,
                                )
                    ot = ev_pool.tile([P, 4], E_DT if out16 else F32, tag="ot4", name="ot4")
                    nc.vector.tensor_copy(ot[:], s_full[:, 0:4])
                    nc.sync.dma_start(out_d[0][:, 0:4], ot[:])
                    return
                if probe == "dma":
                    ot = ev_pool.tile([P, 4], E_DT if out16 else F32, tag="ot4", name="ot4")
                    nc.vector.tensor_copy(ot[:], secs[0][:, 0:4])
                    nc.sync.dma_start(out_d[0][:, 0:4], ot[:])
                    return

                last_e = None
                for s in range(SLOTS):
                    nkt = nkt_slots[s]
                    sec_t = secs[s]
                    qt_t = sec_t[:, 0:NQ]
                    kt_t = sec_t[:, NQ : NQ + nkt * P]
                    v_t = vsecs[s]

                    o_ps = [
                        o_pool.tile([P, QCH], F32, tag="o", name=f"o{qc}")
                        for qc in range(NQC)
                    ]
                    den_ps = [
                        d_pool.tile([P, QCH], F32, tag="den", name=f"den{qc}")
                        for qc in range(NQC)
                    ]

                    den_end = bool(os.environ.get("ATTN_DEN_END"))
                    o_first = not os.environ.get("ATTN_DEN_FIRST")

                    def phase2(kti, e_t):
                        def den_mms():
                            if probe != "noden" and not den_end:
                                for qc in range(NQC):
                                    nc.tensor.matmul(
                                        den_ps[qc][:],
                                        ones_r,
                                        e_t[:, qc * QCH : (qc + 1) * QCH],
                                        start=(kti == 0),
                                        stop=(kti == nkt - 1),
                                    )

                        if not o_first:
                            den_mms()
                        for qc in range(NQC):
                            nc.tensor.matmul(
                                o_ps[qc][:],
                                v_t[:, kti * D : (kti + 1) * D],
                                e_t[:, qc * QCH : (qc + 1) * QCH],
                                start=(kti == 0),
                                stop=(kti == nkt - 1),
                            )
                        if o_first:
                            den_mms()

                    prev = None
                    e_hist = []
                    for kti in range(nkt):
                        if s16:
                            s_full = s_pool.tile([P, NQ], E_DT, tag="s", name="s_ps")
                            s_chunks = [s_full]
                            nc.tensor.matmul(
                                s_full[:],
                                kt_t[:, kti * P : (kti + 1) * P],
                                qt_t[:],
                                start=True,
                                stop=True,
                            )
                        elif narrow_s:
                            s_chunks = [
                                s_pool.tile([P, QCH], F32, tag="s", name="s_ps")
                                for _ in range(NQC)
                            ]
                            for qc in range(NQC):
                                nc.tensor.matmul(
                                    s_chunks[qc][:],
                                    kt_t[:, kti * P : (kti + 1) * P],
                                    qt_t[:, qc * QCH : (qc + 1) * QCH],
                                    start=True,
                                    stop=True,
                                )
                        else:
                            s_full = s_pool.tile([P, NQ], F32, tag="s", name="s_ps")
                            s_chunks = [
                                s_full[:, qc * QCH : (qc + 1) * QCH]
                                for qc in range(NQC)
                            ]
                            for qc in range(NQC):
                                nc.tensor.matmul(
                                    s_chunks[qc],
                                    kt_t[:, kti * P : (kti + 1) * P],
                                    qt_t[:, qc * QCH : (qc + 1) * QCH],
                                    start=True,
                                    stop=True,
                                )
                        e_t = e_pool.tile([P, NQ], E_DT, tag="e", name="e_t")
                        if probe == "s":
                            nc.vector.tensor_copy(e_t[:, 0:4], s_chunks[0][:, 0:4])
                            last_e = e_t
                            continue
                        if narrow_s:
                            for qc in range(NQC):
                                nc.scalar.activation(
                                    e_t[:, qc * QCH : (qc + 1) * QCH],
                                    s_chunks[qc][:],
                                    mybir.ActivationFunctionType.Exp,
                                    bias=bias_all[:, s, kti : kti + 1],
                                    scale=SCALE,
                                )
                        else:
                            nc.scalar.activation(
                                e_t[:],
                                s_full[:],
                                mybir.ActivationFunctionType.Exp,
                                bias=bias_all[:, s, kti : kti + 1],
                                scale=SCALE,
                            )
                        if probe == "se":
                            last_e = e_t
                            continue
                        # software-pipeline phase 2 one k-tile behind so the PE
                        # never waits on the exp of the tile it just produced
                        e_hist.append((kti, e_t))
                        if prev is not None:
                            phase2(*prev)
                        prev = (kti, e_t)
                    if probe in ("s", "se"):
                        ot = ev_pool.tile([P, NQ], F32, tag="ot", name="ot")
                        nc.vector.tensor_copy(ot[:, 0:4], last_e[:, 0:4])
                        ENGS[s % 3].dma_start(out_d[s][:, 0:4], ot[:, 0:4])
                        continue
                    phase2(*prev)
                    if den_end and probe != "noden":
                        for qc in range(NQC):
                            for kti, e_t in e_hist:
                                nc.tensor.matmul(
                                    den_ps[qc][:],
                                    ones_r,
                                    e_t[:, qc * QCH : (qc + 1) * QCH],
                                    start=(kti == 0),
                                    stop=(kti == nkt - 1),
                                )

                    act_evict = bool(os.environ.get("ATTN_ACT_EVICT"))
                    ot = ev_pool.tile([P, NQ], E_DT if out16 else F32, tag="ot", name="ot")
                    for qc in range(NQC):
                        if probe in ("noden", "nonorm"):
                            nc.vector.tensor_copy(
                                ot[:, qc * QCH : (qc + 1) * QCH], o_ps[qc][:]
                            )
                        elif act_evict:
                            o_sb = ev_pool.tile([P, QCH], F32, tag="osb", name="o_sb")
                            nc.scalar.copy(o_sb[:], o_ps[qc][:])
                            rc = ev_pool.tile([P, QCH], F32, tag="rc", name="rc")
                            nc.vector.reciprocal_approx_fast(rc[:], den_ps[qc][:])
                            nc.vector.tensor_mul(
                                ot[:, qc * QCH : (qc + 1) * QCH], o_sb[:], rc[:]
                            )
                        else:
                            rc = ev_pool.tile([P, QCH], F32, tag="rc", name="rc")
                            nc.vector.reciprocal_approx_fast(rc[:], den_ps[qc][:])
                            nc.vector.tensor_mul(
                                ot[:, qc * QCH : (qc + 1) * QCH], o_ps[qc][:], rc[:]
                            )
                    if not os.environ.get("ATTN_SLOT_ST"):
                        for qc in range(NQC):
                            ENGS[(s + qc) % 3].dma_start(
                                out_d[s][:, qc * QCH : (qc + 1) * QCH],
                                ot[:, qc * QCH : (qc + 1) * QCH],
                            )
                    else:
                        ENGS[s % 3].dma_start(out_d[s], ot[:])

            if reps == 1:
                body()
            else:
                with tc.For_i(
                    0,
                    reps,
                    1,
                    hint_engines=(
                        mybir.EngineType.PE,
                        mybir.EngineType.Activation,
                        mybir.EngineType.SP,
                        mybir.EngineType.DVE,
                    ),
                    staggered_reset=bool(os.environ.get("ATTN_STAGGER")),
                ):
                    body()

    nc.compile()
    return nc


def _plan(valid_lens):
    """Sort batches by k-tile count, deal into [slot, core] grid.

    Returns (assign [SLOTS, N_CORES] batch indices, nkt_slots tuple).
    Slot j of every core runs with the same static k-tile count
    (the max over that slot's batches = first element, sorted desc).
    """
    valid = np.asarray(valid_lens).astype(np.int64)
    nkt = (valid + P - 1) // P  # in 1..8
    order = np.argsort(-nkt, kind="stable")
    assign = order.reshape(SLOTS, N_CORES)
    nkt_slots = tuple(int(nkt[assign[j, 0]]) for j in range(SLOTS))
    return assign, nkt_slots


def _round_fp32r(x):
    """Round fp32 to the fp32r (e8m11) grid: RNE at mantissa bit 12."""
    if MM_DT != mybir.dt.float32r:
        return np.ascontiguousarray(x, np.float32)
    u = np.ascontiguousarray(x, np.float32).view(np.uint32).copy()
    lsb = (u >> 12) & 1
    u = (u + 0x7FF + lsb) & 0xFFFFF000
    return u.view(np.float32)


def _prep_inputs(queries, keys, values, valid_lens, assign, nkt_slots):
    """Host-side layout prep + shard into per-core input maps."""
    q = np.ascontiguousarray(queries, dtype=np.float32)
    k = np.ascontiguousarray(keys, dtype=np.float32)
    v = np.ascontiguousarray(values, dtype=np.float32)
    valid = np.asarray(valid_lens).astype(np.int64)

    if os.environ.get("ATTN_QK32R"):
        qT = _round_fp32r(q.transpose(0, 2, 1))  # [B, D, NQ]
        kT = _round_fp32r(k.transpose(0, 2, 1))  # [B, D, NK]
    else:
        qT = np.ascontiguousarray(q.transpose(0, 2, 1)).astype(np.float16)
        kT = np.ascontiguousarray(k.transpose(0, 2, 1)).astype(np.float16)
    # v_prep[b, p, t*D + d] = v[b, t*P + p, d]  (k-tile index t, within-tile p)
    v_prep = np.ascontiguousarray(
        v.reshape(B, KT_MAX, P, D).transpose(0, 2, 1, 3).reshape(B, P, KT_MAX * D)
    ).astype(np.float16)
    key_idx = np.arange(KT_MAX)[:, None] * P + np.arange(P)[None, :]  # [t, p]
    bias = np.where(
        key_idx[None, :, :] < valid[:, None, None], 0.0, MASK_BIAS
    ).astype(np.float32)  # [B, t, p]
    bias = np.ascontiguousarray(bias.transpose(0, 2, 1))  # [B, P, KT_MAX]

    in_maps = []
    ones = np.ones((P, P), np.float16)
    for c in range(N_CORES):
        parts = []
        vparts = []
        bias_core = np.empty((P, SLOTS, KT_MAX), np.float32)
        for s in range(SLOTS):
            b = assign[s, c]
            nkt = nkt_slots[s]
            parts.append(qT[b])
            parts.append(kT[b][:, : nkt * P])
            vparts.append(v_prep[b][:, : nkt * D])
            bias_core[:, s, :] = bias[b]
        blob = np.ascontiguousarray(np.concatenate(parts, axis=1))
        vblob = np.ascontiguousarray(np.concatenate(vparts, axis=1))
        in_maps.append(
            {"blob": blob, "vblob": vblob, "bias": bias_core, "ones": ones}
        )
    return in_maps


def _gather_output(results, assign):
    out = np.empty((B, NQ, D), np.float32)
    for c in range(N_CORES):
        ot = results[c]["out_t"]  # [SLOTS, P(d), NQ]
        if ot.dtype != np.float32:
            ot = ot.astype(np.float32)
        for j in range(SLOTS):
            out[assign[j, c]] = ot[j].T
    return out


_PROGRAM_CACHE = {}


def _get_program(nkt_slots, reps=1, probe=""):
    cfg = (
        os.environ.get("ATTN_NARROW_S", ""),
        os.environ.get("ATTN_DEN_END", ""),
        os.environ.get("ATTN_QK32R", ""),
        os.environ.get("ATTN_E_BUFS", ""),
        os.environ.get("ATTN_SB1", ""),
        os.environ.get("ATTN_SPLIT3", ""),
        os.environ.get("ATTN_S16", ""),
        os.environ.get("ATTN_OUT32", ""),
        os.environ.get("ATTN_STAGGER", ""),
        os.environ.get("ATTN_ACT_EVICT", ""),
        os.environ.get("ATTN_DEN_FIRST", ""),
        os.environ.get("ATTN_SLOT_ST", ""),
    )
    key = (nkt_slots, reps, MM_DT, probe, cfg)
    if key not in _PROGRAM_CACHE:
        _PROGRAM_CACHE[key] = build_program(nkt_slots, reps=reps, probe=probe)
    return _PROGRAM_CACHE[key]


def kernel(queries, keys, values, valid_lens):
    assign, nkt_slots = _plan(valid_lens)
    in_maps = _prep_inputs(queries, keys, values, valid_lens, assign, nkt_slots)
    nc = _get_program(nkt_slots, reps=1)
    res = bass_utils.run_bass_kernel_spmd(nc, in_maps, core_ids=list(range(N_CORES)))
    return _gather_output(res.results, assign)


def run_with_reps(queries, keys, values, valid_lens, reps, probe=""):
    """Run the kernel with the whole per-core body repeated `reps` times on
    device (for wall-clock-delta timing). Returns the gathered output."""
    assign, nkt_slots = _plan(valid_lens)
    in_maps = _prep_inputs(queries, keys, values, valid_lens, assign, nkt_slots)
    nc = _get_program(nkt_slots, reps=reps, probe=probe)
    res = bass_utils.run_bass_kernel_spmd(nc, in_maps, core_ids=list(range(N_CORES)))
    return _gather_output(res.results, assign)
